# revision 1
# baseline (speedup 1.0000x reference)
"""Trainium2 Bass kernel for nn_ConstraintOptimizer (arc-length projection).

Contract: kernel(**inputs) takes FULL unsharded inputs
  selected_traj [1024, 80, 3] f32, road_points [1024, 16, 256, 3] f32,
  road_mask [1024, 16, 256] bool
and returns the FULL output [1024, 80, 3] f32.

Sharding: pure data parallel, N=1024 samples split across 8 NeuronCores
(128 samples/core). Each core processes 4096 candidate polylines
(128 samples x 16 boundaries x 2 directions) in 32 tiles of 128
candidates; within a tile, partition p = d*64 + s*16 + b (4 samples).

Algorithm (gather-free; this toolchain cannot compile any indirect
gather): for each candidate polyline, the arc-length resample is
  proj(s_t) = p_first + sum_i sv_i * clip((s_t - c_i) / len_i, 0, 1)
which is exact for s in [0, total] including all ragged-mask edge
cases, so no searchsorted / per-partition gather is needed. The
backward direction is realized by a negative-stride DMA of the full
row plus masking (phantom zero-length prefix contributes nothing).
The per-sample argmin over 32 candidates uses a PE one-hot matmul to
select the winning projection across partitions.
"""

import sys

for _p in ("/opt/trn_rl_repo",):
    if _p not in sys.path:
        sys.path.insert(0, _p)

import contextlib

import numpy as np

import concourse.bass as bass
import concourse.mybir as mybir
from concourse import tile
from concourse.bass_utils import run_bass_kernel_spmd

F32 = mybir.dt.float32
U8 = mybir.dt.uint8
OP = mybir.AluOpType
AF = mybir.ActivationFunctionType

N, NB, NP, T = 1024, 16, 256, 80
NS_SEG = NP - 1           # 255 segments
NCORES = 8
NS = N // NCORES          # samples per core = 128
CPS = NB * 2              # candidates per sample = 32
NCAND = NS * CPS
TILE = 128
SPT = TILE // CPS         # samples per tile = 4
NTILES = NCAND // TILE    # 32
TH = T // 2               # t-half for the big density tiles
EPS_LEN = 1e-9
EPS_DD = 1e-12
BIG = 3.0e38


def _bcast_mid(ap, n):
    """[P, K] -> [P, n, K] with stride-0 middle dim."""
    assert len(ap.ap) == 2
    return bass.AP(ap.tensor, ap.offset, [ap.ap[0], [0, n], ap.ap[1]])


def _legalize_multiwait(nc):
    """This walrus build accepts only one semaphore wait per instruction.
    Split extra waits into standalone event-semaphore waits inserted just
    before, on the same engine stream (semantically identical: the engine
    stalls on the standalone waits first)."""
    counter = [0]
    for fn in nc.m.functions:
        for bb in fn.blocks:
            insts = bb.instructions
            i = 0
            while i < len(insts):
                ins = insts[i]
                si = ins.sync_info
                if (si is not None and len(si.on_wait) > 1
                        and all(w.sync_type == "semaphore" and w.wait_reg is None
                                for w in si.on_wait)):
                    waits = list(si.on_wait)
                    pre = []
                    for w in waits[:-1]:
                        ev = mybir.InstEventSemaphore(
                            name=f"LGW-{counter[0]}", engine=ins.engine,
                            sync_info=mybir.SyncInfo(on_wait=[w], on_update=[]))
                        counter[0] += 1
                        nc.inst_map[ev.name] = ev
                        pre.append(ev)
                    ins.sync_info = mybir.SyncInfo(on_wait=[waits[-1]],
                                                  on_update=list(si.on_update))
                    insts[i:i] = pre
                    i += len(pre)
                i += 1
    return counter[0]


def build_program():
    nc = bass.Bass()

    rp = nc.dram_tensor("rp", [NS, NB, NP, 3], F32, kind="ExternalInput")
    msk = nc.dram_tensor("msk", [NS, NB, NP], U8, kind="ExternalInput")
    tr = nc.dram_tensor("tr", [NS, T, 3], F32, kind="ExternalInput")
    # host constants
    sel4 = nc.dram_tensor("sel4", [SPT, TILE], F32, kind="ExternalInput")
    i4 = nc.dram_tensor("i4", [SPT, SPT], F32, kind="ExternalInput")
    qp = nc.dram_tensor("qp", [TILE, 1], F32, kind="ExternalInput")
    selt = nc.dram_tensor("selt", [TILE, SPT], F32, kind="ExternalInput")
    io32 = nc.dram_tensor("io32", [SPT, CPS], F32, kind="ExternalInput")
    out = nc.dram_tensor("out", [NS, T * 3], F32, kind="ExternalOutput")

    with tile.TileContext(nc) as tc:
        _body(nc, tc, rp, msk, tr, sel4, i4, qp, selt, io32, out)
    _legalize_multiwait(nc)
    return nc


def _body(nc, tc, rp, msk, tr, sel4, i4, qp, selt, io32, out):
    ctx = contextlib.ExitStack()
    with ctx:
        sb = ctx.enter_context(tc.tile_pool(name="sb", bufs=2))
        sbc = ctx.enter_context(tc.tile_pool(name="sbc", bufs=1))
        sbh = ctx.enter_context(tc.tile_pool(name="sbh", bufs=2))
        ps = ctx.enter_context(tc.tile_pool(name="ps", bufs=2, space="PSUM"))

        sel4_s = sbc.tile([SPT, TILE], F32, tag="sel4")
        nc.sync.dma_start(out=sel4_s[:], in_=sel4[:])
        i4_s = sbc.tile([SPT, SPT], F32, tag="i4")
        nc.sync.dma_start(out=i4_s[:], in_=i4[:])
        qp_s = sbc.tile([TILE, 1], F32, tag="qp")
        nc.sync.dma_start(out=qp_s[:], in_=qp[:])
        selt_s = sbc.tile([TILE, SPT], F32, tag="selt")
        nc.sync.dma_start(out=selt_s[:], in_=selt[:])
        io32_s = sbc.tile([SPT, CPS], F32, tag="io32")
        nc.sync.dma_start(out=io32_s[:], in_=io32[:])

        for ti in range(NTILES):
            n0 = ti * SPT

            # ---------- load ----------
            PT = sb.tile([TILE, NP * 3], F32, tag="PT")
            nc.sync.dma_start(out=PT[0:TILE // 2, :], in_=rp[n0:n0 + SPT])
            nc.sync.dma_start(out=PT[TILE // 2:TILE, :],
                              in_=rp[n0:n0 + SPT, :, ::-1, :])
            MU = sb.tile([TILE, NP], U8, tag="MU")
            nc.sync.dma_start(out=MU[0:TILE // 2, :], in_=msk[n0:n0 + SPT])
            nc.sync.dma_start(out=MU[TILE // 2:TILE, :],
                              in_=msk[n0:n0 + SPT, :, ::-1])
            TR4 = sb.tile([SPT, T * 3], F32, tag="TR4")
            nc.sync.dma_start(out=TR4[:],
                              in_=tr[n0:n0 + SPT].rearrange("s t c -> s (t c)"))

            def comp(apx, c, w=NS_SEG, s=3):
                a = apx[:]
                return bass.AP(a.tensor, a.offset + c, [a.ap[0], [s, w]])

            # ---------- masks / points ----------
            MF = sb.tile([TILE, NP], F32, tag="MF")
            H2 = TILE // 2
            nc.vector.tensor_copy(out=MF[0:H2, :], in_=MU[0:H2, :])
            nc.vector.tensor_copy(out=MF[H2:TILE, :], in_=MU[H2:TILE, :])
            MF3 = sb.tile([TILE, NP * 3], F32, tag="MF3")
            for c in range(3):
                nc.vector.tensor_copy(out=comp(MF3, c, NP), in_=MF[:])
            PM = sb.tile([TILE, NP * 3], F32, tag="PM")
            nc.vector.tensor_tensor(out=PM[0:H2, :], in0=PT[0:H2, :],
                                    in1=MF3[0:H2, :], op=OP.mult)
            nc.vector.tensor_tensor(out=PM[H2:TILE, :], in0=PT[H2:TILE, :],
                                    in1=MF3[H2:TILE, :], op=OP.mult)

            SM = sb.tile([TILE, NS_SEG], F32, tag="SM")
            nc.vector.tensor_tensor(out=SM[:], in0=MF[:, 1:NP], in1=MF[:, 0:NP - 1],
                                    op=OP.mult)
            SV = sb.tile([TILE, NS_SEG * 3], F32, tag="SV")
            nc.vector.tensor_tensor(out=SV[:], in0=PM[:, 3:NP * 3],
                                    in1=PM[:, 0:NS_SEG * 3], op=OP.subtract)
            SM3 = sb.tile([TILE, NS_SEG * 3], F32, tag="SM3")
            for c in range(3):
                nc.vector.tensor_copy(out=comp(SM3, c), in_=SM[:])
            nc.vector.tensor_tensor(out=SV[:], in0=SV[:], in1=SM3[:], op=OP.mult)

            # ---------- lengths / cumulative arc ----------
            D2 = sb.tile([TILE, NS_SEG], F32, tag="D2")
            TMP = sb.tile([TILE, NS_SEG], F32, tag="TMP")
            nc.vector.tensor_tensor(out=D2[:], in0=comp(SV, 0), in1=comp(SV, 0), op=OP.mult)
            nc.vector.tensor_tensor(out=TMP[:], in0=comp(SV, 1), in1=comp(SV, 1), op=OP.mult)
            nc.vector.tensor_tensor(out=D2[:], in0=D2[:], in1=TMP[:], op=OP.add)
            nc.vector.tensor_tensor(out=TMP[:], in0=comp(SV, 2), in1=comp(SV, 2), op=OP.mult)
            nc.vector.tensor_tensor(out=D2[:], in0=D2[:], in1=TMP[:], op=OP.add)
            LEN = sb.tile([TILE, NS_SEG], F32, tag="LEN")
            nc.scalar.activation(out=LEN[:], in_=D2[:], func=AF.Sqrt)
            nc.vector.tensor_scalar(out=LEN[:], in0=LEN[:], scalar1=EPS_LEN,
                                    scalar2=None, op0=OP.max)
            nc.vector.tensor_tensor(out=LEN[:], in0=LEN[:], in1=SM[:], op=OP.mult)
            RLEN = sb.tile([TILE, NS_SEG], F32, tag="RLEN")
            TMP2 = sb.tile([TILE, NS_SEG], F32, tag="TMP2")
            nc.vector.tensor_scalar(out=TMP2[:], in0=LEN[:], scalar1=EPS_LEN,
                                    scalar2=None, op0=OP.max)
            nc.vector.reciprocal(out=RLEN[:], in_=TMP2[:])
            RDD = sb.tile([TILE, NS_SEG], F32, tag="RDD")
            nc.vector.tensor_scalar(out=TMP2[:], in0=D2[:], scalar1=EPS_DD,
                                    scalar2=None, op0=OP.max)
            nc.vector.reciprocal(out=RDD[:], in_=TMP2[:])

            C = sb.tile([TILE, NP], F32, tag="C")
            nc.vector.memset(C[:, 0:1], 0.0)
            nc.vector.tensor_tensor_scan(out=C[:, 1:NP], data0=LEN[:], data1=LEN[:],
                                         initial=0.0, op0=OP.add, op1=OP.bypass)
            TOT = C[:, NP - 1:NP]

            # ---------- p_first = sum_k pts_masked[k] * (m_k - m_{k-1}) ----------
            FM = sb.tile([TILE, NP], F32, tag="FM")
            nc.vector.tensor_copy(out=FM[:, 0:1], in_=MF[:, 0:1])
            nc.vector.tensor_tensor(out=FM[:, 1:NP], in0=MF[:, 1:NP],
                                    in1=MF[:, 0:NP - 1], op=OP.subtract)
            PF = sb.tile([TILE, 3], F32, tag="PF")
            SCR = sb.tile([TILE, NP], F32, tag="SCR")
            for c in range(3):
                nc.vector.tensor_tensor(out=SCR[:], in0=comp(PM, c, NP), in1=FM[:],
                                        op=OP.mult)
                nc.vector.tensor_reduce(out=PF[:, c:c + 1], in_=SCR[:],
                                        axis=mybir.AxisListType.X, op=OP.add)

            # ---------- trajectory (replicated to all 128 partitions) ----------
            TRP = ps.tile([TILE, T * 3], F32, tag="TRP")
            nc.tensor.matmul(TRP[:], lhsT=sel4_s[:], rhs=TR4[:], start=True, stop=True)
            TRR = sb.tile([TILE, T * 3], F32, tag="TRR")
            nc.vector.tensor_copy(out=TRR[:], in_=TRP[:])

            TSG = sb.tile([TILE, (T - 1) * 3], F32, tag="TSG")
            nc.vector.tensor_tensor(out=TSG[:], in0=TRR[:, 3:T * 3],
                                    in1=TRR[:, 0:(T - 1) * 3], op=OP.subtract)
            TD2 = sb.tile([TILE, T - 1], F32, tag="TD2")
            TT2 = sb.tile([TILE, T - 1], F32, tag="TT2")
            nc.vector.tensor_tensor(out=TD2[:], in0=comp(TSG, 0, T - 1), in1=comp(TSG, 0, T - 1), op=OP.mult)
            nc.vector.tensor_tensor(out=TT2[:], in0=comp(TSG, 1, T - 1), in1=comp(TSG, 1, T - 1), op=OP.mult)
            nc.vector.tensor_tensor(out=TD2[:], in0=TD2[:], in1=TT2[:], op=OP.add)
            nc.vector.tensor_tensor(out=TT2[:], in0=comp(TSG, 2, T - 1), in1=comp(TSG, 2, T - 1), op=OP.mult)
            nc.vector.tensor_tensor(out=TD2[:], in0=TD2[:], in1=TT2[:], op=OP.add)
            TLN = sb.tile([TILE, T - 1], F32, tag="TLN")
            nc.scalar.activation(out=TLN[:], in_=TD2[:], func=AF.Sqrt)
            L = sb.tile([TILE, T], F32, tag="L")
            nc.vector.memset(L[:, 0:1], 0.0)
            nc.vector.tensor_tensor_scan(out=L[:, 1:T], data0=TLN[:], data1=TLN[:],
                                         initial=0.0, op0=OP.add, op1=OP.bypass)

            # ---------- entry projection ----------
            PA = sb.tile([TILE, NS_SEG * 3], F32, tag="PA")
            for c in range(3):
                nc.vector.tensor_scalar(out=comp(PA, c), in0=comp(PM, c),
                                        scalar1=-1.0, scalar2=TRR[:, c:c + 1],
                                        op0=OP.mult, op1=OP.add)
            DOT = sb.tile([TILE, NS_SEG], F32, tag="DOT")
            nc.vector.tensor_tensor(out=DOT[:], in0=comp(PA, 0), in1=comp(SV, 0), op=OP.mult)
            nc.vector.tensor_tensor(out=TMP[:], in0=comp(PA, 1), in1=comp(SV, 1), op=OP.mult)
            nc.vector.tensor_tensor(out=DOT[:], in0=DOT[:], in1=TMP[:], op=OP.add)
            nc.vector.tensor_tensor(out=TMP[:], in0=comp(PA, 2), in1=comp(SV, 2), op=OP.mult)
            nc.vector.tensor_tensor(out=DOT[:], in0=DOT[:], in1=TMP[:], op=OP.add)
            T0 = sb.tile([TILE, NS_SEG], F32, tag="T0")
            nc.vector.tensor_tensor(out=T0[:], in0=DOT[:], in1=RDD[:], op=OP.mult)
            nc.vector.tensor_scalar(out=T0[:], in0=T0[:], scalar1=0.0, scalar2=1.0,
                                    op0=OP.max, op1=OP.min)
            D2Q = sb.tile([TILE, NS_SEG], F32, tag="D2Q")
            first = True
            for c in range(3):
                nc.vector.tensor_tensor(out=TMP[:], in0=T0[:], in1=comp(SV, c), op=OP.mult)
                nc.vector.tensor_tensor(out=TMP[:], in0=comp(PA, c), in1=TMP[:], op=OP.subtract)
                nc.vector.tensor_tensor(out=TMP[:], in0=TMP[:], in1=TMP[:], op=OP.mult)
                if first:
                    nc.vector.tensor_copy(out=D2Q[:], in_=TMP[:])
                    first = False
                else:
                    nc.vector.tensor_tensor(out=D2Q[:], in0=D2Q[:], in1=TMP[:], op=OP.add)
            nc.vector.tensor_scalar(out=TMP[:], in0=SM[:], scalar1=-BIG, scalar2=BIG,
                                    op0=OP.mult, op1=OP.add)
            nc.vector.tensor_tensor(out=D2Q[:], in0=D2Q[:], in1=TMP[:], op=OP.add)
            ENT = sb.tile([TILE, NS_SEG], F32, tag="ENT")
            nc.vector.tensor_tensor(out=ENT[:], in0=T0[:], in1=LEN[:], op=OP.mult)
            nc.vector.tensor_tensor(out=ENT[:], in0=ENT[:], in1=C[:, 0:NS_SEG], op=OP.add)
            MINV = sb.tile([TILE, 1], F32, tag="MINV")
            nc.vector.tensor_reduce(out=MINV[:], in_=D2Q[:], axis=mybir.AxisListType.X,
                                    op=OP.min)
            EQM = sb.tile([TILE, NS_SEG], F32, tag="EQM")
            nc.vector.tensor_scalar(out=EQM[:], in0=D2Q[:], scalar1=MINV[:],
                                    scalar2=None, op0=OP.is_equal)
            nc.vector.tensor_scalar(out=EQM[:], in0=EQM[:], scalar1=-BIG, scalar2=BIG,
                                    op0=OP.mult, op1=OP.add)
            nc.vector.tensor_tensor(out=EQM[:], in0=EQM[:], in1=ENT[:], op=OP.add)
            ENTRY = sb.tile([TILE, 1], F32, tag="ENTRY")
            nc.vector.tensor_reduce(out=ENTRY[:], in_=EQM[:], axis=mybir.AxisListType.X,
                                    op=OP.min)

            # ---------- targets ----------
            S = sb.tile([TILE, T], F32, tag="S")
            nc.vector.tensor_scalar(out=S[:], in0=L[:], scalar1=ENTRY[:], scalar2=TOT,
                                    op0=OP.add, op1=OP.min)

            # ---------- dense arc interpolation (exact, gather-free) ----------
            # proj_c(t) = PF_c + sum_i sv_ic * clip((s_t - c_i) * rlen_i, 0, 1)
            TQ = T // 4
            PRJ = sb.tile([TILE, T * 3], F32, tag="PRJ")
            for h in range(4):
                CL = sbh.tile([TILE, TQ, NS_SEG], F32, tag="CL")
                sh = S[:, h * TQ:(h + 1) * TQ]
                s_b = sh.to_broadcast([TILE, TQ, NS_SEG])
                c_b = _bcast_mid(C[:, 0:NS_SEG], TQ)
                rl_b = _bcast_mid(RLEN[:], TQ)
                # CL = clip((s - c) * rlen, 0, 1)
                nc.vector.scalar_tensor_tensor(out=CL[:], in0=c_b, scalar=-1.0,
                                               in1=s_b, op0=OP.mult, op1=OP.add)
                nc.vector.tensor_tensor(out=CL[:], in0=CL[:], in1=rl_b, op=OP.mult)
                nc.vector.tensor_scalar(out=CL[:], in0=CL[:], scalar1=0.0,
                                        scalar2=1.0, op0=OP.max, op1=OP.min)
                for c in range(3):
                    PRD = sbh.tile([TILE, TQ, NS_SEG], F32, tag="PRD")
                    sv_b = _bcast_mid(comp(SV, c), TQ)
                    # product on GPSIMD, overlapped with DVE's CL/reduce work
                    nc.gpsimd.tensor_tensor(out=PRD[:], in0=CL[:], in1=sv_b, op=OP.mult)
                    pr = bass.AP(PRJ[:].tensor, PRJ[:].offset + h * TQ * 3 + c,
                                 [PRJ[:].ap[0], [3, TQ]])
                    nc.vector.tensor_reduce(out=pr, in_=PRD[:],
                                            axis=mybir.AxisListType.X, op=OP.add)
            for c in range(3):
                nc.vector.tensor_scalar(out=comp(PRJ, c, T), in0=comp(PRJ, c, T),
                                        scalar1=PF[:, c:c + 1], scalar2=None,
                                        op0=OP.add)

            # ---------- cost ----------
            DTMP = sb.tile([TILE, T], F32, tag="DTMP")
            D2T = sb.tile([TILE, T], F32, tag="D2T")
            first = True
            for c in range(3):
                nc.vector.tensor_tensor(out=DTMP[:], in0=comp(TRR, c, T),
                                        in1=comp(PRJ, c, T), op=OP.subtract)
                nc.vector.tensor_tensor(out=DTMP[:], in0=DTMP[:], in1=DTMP[:], op=OP.mult)
                if first:
                    nc.vector.tensor_copy(out=D2T[:], in_=DTMP[:])
                    first = False
                else:
                    nc.vector.tensor_tensor(out=D2T[:], in0=D2T[:], in1=DTMP[:], op=OP.add)
            DIST = sb.tile([TILE, T], F32, tag="DIST")
            COST = sb.tile([TILE, 1], F32, tag="COST")
            nc.scalar.activation(out=DIST[:], in_=D2T[:], func=AF.Sqrt,
                                 accum_out=COST[:])

            # ---------- per-tile best-candidate selection ----------
            # costs to [4 samples, 32 cands] layout (q = d*16 + b)
            CBT = sb.tile([SPT, CPS], F32, tag="CBT")
            for d in range(2):
                nc.sync.dma_start(out=CBT[0:SPT, d * 16:(d + 1) * 16],
                                  in_=COST[d * 64:(d + 1) * 64, 0:1])
            MN4 = sb.tile([SPT, 1], F32, tag="MN4")
            nc.vector.tensor_reduce(out=MN4[:], in_=CBT[:], axis=mybir.AxisListType.X,
                                    op=OP.min)
            EQ4 = sb.tile([SPT, CPS], F32, tag="EQ4")
            nc.vector.tensor_scalar(out=EQ4[:], in0=CBT[:], scalar1=MN4[:],
                                    scalar2=None, op0=OP.is_equal)
            nc.vector.tensor_scalar(out=EQ4[:], in0=EQ4[:], scalar1=-BIG, scalar2=BIG,
                                    op0=OP.mult, op1=OP.add)
            nc.vector.tensor_tensor(out=EQ4[:], in0=EQ4[:], in1=io32_s[:], op=OP.add)
            IDXQ = sb.tile([SPT, 1], F32, tag="IDXQ")
            nc.vector.tensor_reduce(out=IDXQ[:], in_=EQ4[:], axis=mybir.AxisListType.X,
                                    op=OP.min)
            # replicate IDXQ to all 128 partitions: PE ones @ diag(IDXQ)
            DG = sb.tile([SPT, SPT], F32, tag="DG")
            nc.vector.tensor_scalar(out=DG[:], in0=i4_s[:], scalar1=IDXQ[:],
                                    scalar2=None, op0=OP.mult)
            IDXP = ps.tile([TILE, SPT], F32, tag="IDXP")
            nc.tensor.matmul(IDXP[:], lhsT=sel4_s[:], rhs=DG[:], start=True, stop=True)
            IDXR = sb.tile([TILE, SPT], F32, tag="IDXR")
            nc.vector.tensor_copy(out=IDXR[:], in_=IDXP[:])
            # one-hot [128, 4]: partition p selects sample s iff qp[p] == idx[s]
            OH = sb.tile([TILE, SPT], F32, tag="OH")
            nc.vector.tensor_scalar(out=OH[:], in0=IDXR[:], scalar1=qp_s[:],
                                    scalar2=None, op0=OP.is_equal)
            # restrict to the partition's own sample column
            nc.vector.tensor_tensor(out=OH[:], in0=OH[:], in1=selt_s[:], op=OP.mult)
            BPP = ps.tile([SPT, T * 3], F32, tag="BPP")
            nc.tensor.matmul(BPP[:], lhsT=OH[:], rhs=PRJ[:], start=True, stop=True)
            BPS = sb.tile([SPT, T * 3], F32, tag="BPS")
            nc.vector.tensor_copy(out=BPS[:], in_=BPP[:])
            nc.sync.dma_start(out=out[n0:n0 + SPT, :], in_=BPS[:])


_cached = {}


def _consts():
    p = np.arange(TILE)
    # partition p = d*64 + s*16 + b
    sel4 = ((p[None, :] % 64) // NB == np.arange(SPT)[:, None]).astype(np.float32)
    i4 = np.eye(SPT, dtype=np.float32)
    qp = ((p // 64) * NB + p % NB).astype(np.float32)[:, None]
    selt = sel4.T.copy()
    q = np.arange(CPS, dtype=np.float32)
    io32 = np.broadcast_to(q, (SPT, CPS)).copy()
    return dict(sel4=sel4, i4=i4, qp=qp, selt=selt, io32=io32)


def kernel(selected_traj, road_points, road_mask):
    selected_traj = np.asarray(selected_traj)
    road_points = np.asarray(road_points)
    road_mask = np.asarray(road_mask)

    if "nc" not in _cached:
        _cached["nc"] = build_program()
    nc = _cached["nc"]

    consts = _consts()
    in_maps = []
    for c in range(NCORES):
        sl = slice(c * NS, (c + 1) * NS)
        m = {
            "rp": np.ascontiguousarray(road_points[sl], dtype=np.float32),
            "msk": np.ascontiguousarray(road_mask[sl]).astype(np.uint8),
            "tr": np.ascontiguousarray(selected_traj[sl, :, 0:3], dtype=np.float32),
        }
        m.update(consts)
        in_maps.append(m)

    res = run_bass_kernel_spmd(nc, in_maps, list(range(NCORES)),
                               trace=bool(_cached.get("trace", False)))
    _cached["exec_time_ns"] = getattr(res, "exec_time_ns", None)
    outs = [np.asarray(res.results[c]["out"]).reshape(NS, T, 3) for c in range(NCORES)]
    out_pos = np.concatenate(outs, axis=0)

    if selected_traj.shape[-1] > 3:
        out_full = np.concatenate([out_pos, selected_traj[..., 3:]], axis=-1)
    else:
        out_full = out_pos
    return out_full.astype(selected_traj.dtype)



# revision 11
# speedup vs baseline: 1.0734x; 1.0734x over previous
"""Trainium2 Bass kernel for nn_ConstraintOptimizer (arc-length projection).

Contract: kernel(**inputs) takes FULL unsharded inputs
  selected_traj [1024, 80, 3] f32, road_points [1024, 16, 256, 3] f32,
  road_mask [1024, 16, 256] bool
and returns the FULL output [1024, 80, 3] f32.

Sharding: pure data parallel, N=1024 samples split across 8 NeuronCores
(128 samples/core). Each core processes 4096 candidate polylines
(128 samples x 16 boundaries x 2 directions) in 32 tiles of 128
candidates; within a tile, partition p = d*64 + s*16 + b (4 samples).

Algorithm (gather-free; this toolchain cannot compile any indirect
gather): for each candidate polyline, the arc-length resample is
  proj(s_t) = p_first + sum_i sv_i * clip((s_t - c_i) / len_i, 0, 1)
which is exact for s in [0, total] including all ragged-mask edge
cases, so no searchsorted / per-partition gather is needed. The
backward direction is realized by a negative-stride DMA of the full
row plus masking (phantom zero-length prefix contributes nothing).
The per-sample argmin over 32 candidates uses a PE one-hot matmul to
select the winning projection across partitions.
"""

import sys

for _p in ("/opt/trn_rl_repo",):
    if _p not in sys.path:
        sys.path.insert(0, _p)

import contextlib

import numpy as np

import concourse.bass as bass
import concourse.mybir as mybir
from concourse import tile
from concourse.bass_utils import run_bass_kernel_spmd

F32 = mybir.dt.float32
F16 = mybir.dt.float16
U8 = mybir.dt.uint8
OP = mybir.AluOpType
AF = mybir.ActivationFunctionType

N, NB, NP, T = 1024, 16, 256, 80
NS_SEG = NP - 1           # 255 segments
NCORES = 8
NS = N // NCORES          # samples per core = 128
CPS = NB * 2              # candidates per sample = 32
NCAND = NS * CPS
TILE = 128
SPT = TILE // CPS         # samples per tile = 4
NTILES = NCAND // TILE    # 32
TH = T // 2               # t-half for the big density tiles
EPS_LEN = 1e-9
EPS_DD = 1e-12
BIG = 3.0e38


def _bcast_mid(ap, n):
    """[P, K] -> [P, n, K] with stride-0 middle dim."""
    assert len(ap.ap) == 2
    return bass.AP(ap.tensor, ap.offset, [ap.ap[0], [0, n], ap.ap[1]])


def _legalize_multiwait(nc):
    """This walrus build accepts only one semaphore wait per instruction.
    Split extra waits into standalone event-semaphore waits inserted just
    before, on the same engine stream (semantically identical: the engine
    stalls on the standalone waits first)."""
    counter = [0]
    for fn in nc.m.functions:
        for bb in fn.blocks:
            insts = bb.instructions
            i = 0
            while i < len(insts):
                ins = insts[i]
                si = ins.sync_info
                if (si is not None and len(si.on_wait) > 1
                        and all(w.sync_type == "semaphore" and w.wait_reg is None
                                for w in si.on_wait)):
                    waits = list(si.on_wait)
                    pre = []
                    for w in waits[:-1]:
                        ev = mybir.InstEventSemaphore(
                            name=f"LGW-{counter[0]}", engine=ins.engine,
                            sync_info=mybir.SyncInfo(on_wait=[w], on_update=[]))
                        counter[0] += 1
                        nc.inst_map[ev.name] = ev
                        pre.append(ev)
                    ins.sync_info = mybir.SyncInfo(on_wait=[waits[-1]],
                                                  on_update=list(si.on_update))
                    insts[i:i] = pre
                    i += len(pre)
                i += 1
    return counter[0]


def build_program():
    nc = bass.Bass()

    rp = nc.dram_tensor("rp", [NS, NB, NP, 3], F32, kind="ExternalInput")
    msk = nc.dram_tensor("msk", [NS, NB, NP], U8, kind="ExternalInput")
    tr = nc.dram_tensor("tr", [NS, T, 3], F32, kind="ExternalInput")
    # host constants
    sel4 = nc.dram_tensor("sel4", [SPT, TILE], F32, kind="ExternalInput")
    i4 = nc.dram_tensor("i4", [SPT, SPT], F32, kind="ExternalInput")
    qp = nc.dram_tensor("qp", [TILE, 1], F32, kind="ExternalInput")
    selt = nc.dram_tensor("selt", [TILE, SPT], F32, kind="ExternalInput")
    io32 = nc.dram_tensor("io32", [SPT, CPS], F32, kind="ExternalInput")
    out = nc.dram_tensor("out", [NS, T * 3], F32, kind="ExternalOutput")

    with tile.TileContext(nc) as tc, nc.allow_low_precision("fp16 dense interp"):
        _body(nc, tc, rp, msk, tr, sel4, i4, qp, selt, io32, out)
    _legalize_multiwait(nc)
    return nc


def _body(nc, tc, rp, msk, tr, sel4, i4, qp, selt, io32, out):
    ctx = contextlib.ExitStack()
    with ctx:
        sb = ctx.enter_context(tc.tile_pool(name="sb", bufs=2))
        sbc = ctx.enter_context(tc.tile_pool(name="sbc", bufs=1))
        sbh = ctx.enter_context(tc.tile_pool(name="sbh", bufs=2))
        ps = ctx.enter_context(tc.tile_pool(name="ps", bufs=2, space="PSUM"))

        sel4_s = sbc.tile([SPT, TILE], F32, tag="sel4")
        nc.sync.dma_start(out=sel4_s[:], in_=sel4[:])
        i4_s = sbc.tile([SPT, SPT], F32, tag="i4")
        nc.sync.dma_start(out=i4_s[:], in_=i4[:])
        qp_s = sbc.tile([TILE, 1], F32, tag="qp")
        nc.sync.dma_start(out=qp_s[:], in_=qp[:])
        selt_s = sbc.tile([TILE, SPT], F32, tag="selt")
        nc.sync.dma_start(out=selt_s[:], in_=selt[:])
        io32_s = sbc.tile([SPT, CPS], F32, tag="io32")
        nc.sync.dma_start(out=io32_s[:], in_=io32[:])

        for ti in range(NTILES):
            n0 = ti * SPT

            # ---------- load ----------
            PT = sb.tile([TILE, NP * 3], F32, tag="PT")
            nc.sync.dma_start(out=PT[0:TILE // 2, :], in_=rp[n0:n0 + SPT])
            nc.sync.dma_start(out=PT[TILE // 2:TILE, :],
                              in_=rp[n0:n0 + SPT, :, ::-1, :])
            MU = sb.tile([TILE, NP], U8, tag="MU")
            nc.sync.dma_start(out=MU[0:TILE // 2, :], in_=msk[n0:n0 + SPT])
            nc.sync.dma_start(out=MU[TILE // 2:TILE, :],
                              in_=msk[n0:n0 + SPT, :, ::-1])
            TR4 = sb.tile([SPT, T * 3], F32, tag="TR4")
            nc.sync.dma_start(out=TR4[:],
                              in_=tr[n0:n0 + SPT].rearrange("s t c -> s (t c)"))

            def comp(apx, c, w=NS_SEG, s=3):
                a = apx[:]
                return bass.AP(a.tensor, a.offset + c, [a.ap[0], [s, w]])

            # ---------- masks / points ----------
            MF = sb.tile([TILE, NP], F32, tag="MF")
            H2 = TILE // 2
            nc.vector.tensor_copy(out=MF[0:H2, :], in_=MU[0:H2, :])
            nc.vector.tensor_copy(out=MF[H2:TILE, :], in_=MU[H2:TILE, :])
            MF3 = sb.tile([TILE, NP * 3], F32, tag="MF3")
            for c in range(3):
                nc.vector.tensor_copy(out=comp(MF3, c, NP), in_=MF[:])
            PM = sb.tile([TILE, NP * 3], F32, tag="PM")
            nc.vector.tensor_tensor(out=PM[0:H2, :], in0=PT[0:H2, :],
                                    in1=MF3[0:H2, :], op=OP.mult)
            nc.vector.tensor_tensor(out=PM[H2:TILE, :], in0=PT[H2:TILE, :],
                                    in1=MF3[H2:TILE, :], op=OP.mult)

            SM = sb.tile([TILE, NS_SEG], F32, tag="SM")
            nc.vector.tensor_tensor(out=SM[:], in0=MF[:, 1:NP], in1=MF[:, 0:NP - 1],
                                    op=OP.mult)
            SV = sb.tile([TILE, NS_SEG * 3], F32, tag="SV")
            nc.vector.tensor_tensor(out=SV[:], in0=PM[:, 3:NP * 3],
                                    in1=PM[:, 0:NS_SEG * 3], op=OP.subtract)
            SM3 = sb.tile([TILE, NS_SEG * 3], F32, tag="SM3")
            for c in range(3):
                nc.vector.tensor_copy(out=comp(SM3, c), in_=SM[:])
            nc.vector.tensor_tensor(out=SV[:], in0=SV[:], in1=SM3[:], op=OP.mult)

            # ---------- lengths / cumulative arc ----------
            D2 = sb.tile([TILE, NS_SEG], F32, tag="D2")
            TMP = sb.tile([TILE, NS_SEG], F32, tag="TMP")
            nc.vector.tensor_tensor(out=D2[:], in0=comp(SV, 0), in1=comp(SV, 0), op=OP.mult)
            nc.vector.tensor_tensor(out=TMP[:], in0=comp(SV, 1), in1=comp(SV, 1), op=OP.mult)
            nc.vector.tensor_tensor(out=D2[:], in0=D2[:], in1=TMP[:], op=OP.add)
            nc.vector.tensor_tensor(out=TMP[:], in0=comp(SV, 2), in1=comp(SV, 2), op=OP.mult)
            nc.vector.tensor_tensor(out=D2[:], in0=D2[:], in1=TMP[:], op=OP.add)
            LEN = sb.tile([TILE, NS_SEG], F32, tag="LEN")
            nc.scalar.activation(out=LEN[:], in_=D2[:], func=AF.Sqrt)
            nc.vector.tensor_scalar(out=LEN[:], in0=LEN[:], scalar1=EPS_LEN,
                                    scalar2=None, op0=OP.max)
            nc.vector.tensor_tensor(out=LEN[:], in0=LEN[:], in1=SM[:], op=OP.mult)
            RLEN = sb.tile([TILE, NS_SEG], F32, tag="RLEN")
            TMP2 = sb.tile([TILE, NS_SEG], F32, tag="TMP2")
            nc.vector.tensor_scalar(out=TMP2[:], in0=LEN[:], scalar1=EPS_LEN,
                                    scalar2=None, op0=OP.max)
            nc.vector.reciprocal(out=RLEN[:], in_=TMP2[:])
            NRLEN = sb.tile([TILE, NS_SEG], F32, tag="NRLEN")
            nc.vector.tensor_scalar(out=NRLEN[:], in0=RLEN[:], scalar1=-1.0,
                                    scalar2=None, op0=OP.mult)
            RDD = sb.tile([TILE, NS_SEG], F32, tag="RDD")
            nc.vector.tensor_scalar(out=TMP2[:], in0=D2[:], scalar1=EPS_DD,
                                    scalar2=None, op0=OP.max)
            nc.vector.reciprocal(out=RDD[:], in_=TMP2[:])

            # cumsum via log-step shift-adds (the hw scan op costs 11us/tile)
            C = sb.tile([TILE, NP], F32, tag="C")
            CB = sb.tile([TILE, NP], F32, tag="CB")
            nc.vector.memset(C[:, 0:1], 0.0)
            nc.vector.tensor_copy(out=C[:, 1:NP], in_=LEN[:])
            cs, cd = C, CB
            for k in (1, 2, 4, 8, 16, 32, 64, 128):
                nc.vector.tensor_tensor(out=cd[:, k:NP], in0=cs[:, k:NP],
                                        in1=cs[:, 0:NP - k], op=OP.add)
                nc.vector.tensor_copy(out=cd[:, 0:k], in_=cs[:, 0:k])
                cs, cd = cd, cs
            assert cs is C
            TOT = C[:, NP - 1:NP]

            # ---------- p_first = sum_k pts_masked[k] * (m_k - m_{k-1}) ----------
            FM = sb.tile([TILE, NP], F32, tag="FM")
            nc.vector.tensor_copy(out=FM[:, 0:1], in_=MF[:, 0:1])
            nc.vector.tensor_tensor(out=FM[:, 1:NP], in0=MF[:, 1:NP],
                                    in1=MF[:, 0:NP - 1], op=OP.subtract)
            PF = sb.tile([TILE, 3], F32, tag="PF")
            SCR = sb.tile([TILE, NP], F32, tag="SCR")
            for c in range(3):
                nc.vector.tensor_tensor(out=SCR[:], in0=comp(PM, c, NP), in1=FM[:],
                                        op=OP.mult)
                nc.vector.tensor_reduce(out=PF[:, c:c + 1], in_=SCR[:],
                                        axis=mybir.AxisListType.X, op=OP.add)

            # ---------- trajectory (replicated to all 128 partitions) ----------
            TRP = ps.tile([TILE, T * 3], F32, tag="TRP")
            nc.tensor.matmul(TRP[:], lhsT=sel4_s[:], rhs=TR4[:], start=True, stop=True)
            TRR = sb.tile([TILE, T * 3], F32, tag="TRR")
            nc.vector.tensor_copy(out=TRR[:], in_=TRP[:])

            TSG = sb.tile([TILE, (T - 1) * 3], F32, tag="TSG")
            nc.vector.tensor_tensor(out=TSG[:], in0=TRR[:, 3:T * 3],
                                    in1=TRR[:, 0:(T - 1) * 3], op=OP.subtract)
            TD2 = sb.tile([TILE, T - 1], F32, tag="TD2")
            TT2 = sb.tile([TILE, T - 1], F32, tag="TT2")
            nc.vector.tensor_tensor(out=TD2[:], in0=comp(TSG, 0, T - 1), in1=comp(TSG, 0, T - 1), op=OP.mult)
            nc.vector.tensor_tensor(out=TT2[:], in0=comp(TSG, 1, T - 1), in1=comp(TSG, 1, T - 1), op=OP.mult)
            nc.vector.tensor_tensor(out=TD2[:], in0=TD2[:], in1=TT2[:], op=OP.add)
            nc.vector.tensor_tensor(out=TT2[:], in0=comp(TSG, 2, T - 1), in1=comp(TSG, 2, T - 1), op=OP.mult)
            nc.vector.tensor_tensor(out=TD2[:], in0=TD2[:], in1=TT2[:], op=OP.add)
            TLN = sb.tile([TILE, T - 1], F32, tag="TLN")
            nc.scalar.activation(out=TLN[:], in_=TD2[:], func=AF.Sqrt)
            L = sb.tile([TILE, T], F32, tag="L")
            nc.vector.memset(L[:, 0:1], 0.0)
            nc.vector.tensor_tensor_scan(out=L[:, 1:T], data0=TLN[:], data1=TLN[:],
                                         initial=0.0, op0=OP.add, op1=OP.bypass)

            # ---------- entry projection ----------
            PA = sb.tile([TILE, NS_SEG * 3], F32, tag="PA")
            for c in range(3):
                nc.vector.tensor_scalar(out=comp(PA, c), in0=comp(PM, c),
                                        scalar1=-1.0, scalar2=TRR[:, c:c + 1],
                                        op0=OP.mult, op1=OP.add)
            DOT = sb.tile([TILE, NS_SEG], F32, tag="DOT")
            nc.vector.tensor_tensor(out=DOT[:], in0=comp(PA, 0), in1=comp(SV, 0), op=OP.mult)
            nc.vector.tensor_tensor(out=TMP[:], in0=comp(PA, 1), in1=comp(SV, 1), op=OP.mult)
            nc.vector.tensor_tensor(out=DOT[:], in0=DOT[:], in1=TMP[:], op=OP.add)
            nc.vector.tensor_tensor(out=TMP[:], in0=comp(PA, 2), in1=comp(SV, 2), op=OP.mult)
            nc.vector.tensor_tensor(out=DOT[:], in0=DOT[:], in1=TMP[:], op=OP.add)
            T0 = sb.tile([TILE, NS_SEG], F32, tag="T0")
            nc.vector.tensor_tensor(out=T0[:], in0=DOT[:], in1=RDD[:], op=OP.mult)
            nc.vector.tensor_scalar(out=T0[:], in0=T0[:], scalar1=0.0, scalar2=1.0,
                                    op0=OP.max, op1=OP.min)
            D2Q = sb.tile([TILE, NS_SEG], F32, tag="D2Q")
            first = True
            for c in range(3):
                nc.vector.tensor_tensor(out=TMP[:], in0=T0[:], in1=comp(SV, c), op=OP.mult)
                nc.vector.tensor_tensor(out=TMP[:], in0=comp(PA, c), in1=TMP[:], op=OP.subtract)
                nc.vector.tensor_tensor(out=TMP[:], in0=TMP[:], in1=TMP[:], op=OP.mult)
                if first:
                    nc.vector.tensor_copy(out=D2Q[:], in_=TMP[:])
                    first = False
                else:
                    nc.vector.tensor_tensor(out=D2Q[:], in0=D2Q[:], in1=TMP[:], op=OP.add)
            nc.vector.tensor_scalar(out=TMP[:], in0=SM[:], scalar1=-BIG, scalar2=BIG,
                                    op0=OP.mult, op1=OP.add)
            nc.vector.tensor_tensor(out=D2Q[:], in0=D2Q[:], in1=TMP[:], op=OP.add)
            ENT = sb.tile([TILE, NS_SEG], F32, tag="ENT")
            nc.vector.tensor_tensor(out=ENT[:], in0=T0[:], in1=LEN[:], op=OP.mult)
            nc.vector.tensor_tensor(out=ENT[:], in0=ENT[:], in1=C[:, 0:NS_SEG], op=OP.add)
            MINV = sb.tile([TILE, 1], F32, tag="MINV")
            nc.vector.tensor_reduce(out=MINV[:], in_=D2Q[:], axis=mybir.AxisListType.X,
                                    op=OP.min)
            EQM = sb.tile([TILE, NS_SEG], F32, tag="EQM")
            nc.vector.tensor_scalar(out=EQM[:], in0=D2Q[:], scalar1=MINV[:],
                                    scalar2=None, op0=OP.is_equal)
            nc.vector.tensor_scalar(out=EQM[:], in0=EQM[:], scalar1=-BIG, scalar2=BIG,
                                    op0=OP.mult, op1=OP.add)
            nc.vector.tensor_tensor(out=EQM[:], in0=EQM[:], in1=ENT[:], op=OP.add)
            ENTRY = sb.tile([TILE, 1], F32, tag="ENTRY")
            nc.vector.tensor_reduce(out=ENTRY[:], in_=EQM[:], axis=mybir.AxisListType.X,
                                    op=OP.min)

            # ---------- targets ----------
            S = sb.tile([TILE, T], F32, tag="S")
            nc.vector.tensor_scalar(out=S[:], in0=L[:], scalar1=ENTRY[:], scalar2=TOT,
                                    op0=OP.add, op1=OP.min)

            # ---------- dense arc interpolation (exact, gather-free) ----------
            # proj_c(t) = PF_c + sum_i sv_ic * clip((s_t - c_i) * rlen_i, 0, 1)
            # fp32 build of the clip argument runs on GPSIMD; the clamp emits
            # packed fp16 so the 6 multiply/reduce passes hit DVE 2x/4x modes.
            TQ = T // 4
            PRJ = sb.tile([TILE, T * 3], F32, tag="PRJ")
            PRJH = [sb.tile([TILE, T], F32, tag=f"PRJH{c}", name=f"PRJH{c}")
                    for c in range(3)]
            for h in range(4):
                CL = sbh.tile([TILE, TQ, NS_SEG], F32, tag="CL")
                sh = S[:, h * TQ:(h + 1) * TQ]
                s_b = sh.to_broadcast([TILE, TQ, NS_SEG])
                c_b = _bcast_mid(C[:, 0:NS_SEG], TQ)
                # CL = clip((s - c) * rlen, 0, 1) == clip((c - s) * -rlen, ...)
                nrl_b = _bcast_mid(NRLEN[:], TQ)
                nc.gpsimd.tensor_tensor(out=CL[:], in0=c_b, in1=s_b, op=OP.subtract)
                nc.gpsimd.tensor_tensor(out=CL[:], in0=CL[:], in1=nrl_b, op=OP.mult)
                nc.vector.tensor_scalar(out=CL[:], in0=CL[:], scalar1=0.0,
                                        scalar2=1.0, op0=OP.max, op1=OP.min)
                for c in range(3):
                    PRD = sbh.tile([TILE, TQ, NS_SEG], F32, tag="PRD")
                    sv_b = _bcast_mid(comp(SV, c), TQ)
                    nc.vector.tensor_tensor(out=PRD[:], in0=CL[:], in1=sv_b,
                                            op=OP.mult)
                    nc.vector.tensor_reduce(out=PRJH[c][:, h * TQ:(h + 1) * TQ],
                                            in_=PRD[:], axis=mybir.AxisListType.X,
                                            op=OP.add)
            for c in range(3):
                nc.vector.tensor_scalar(out=comp(PRJ, c, T), in0=PRJH[c][:],
                                        scalar1=PF[:, c:c + 1], scalar2=None,
                                        op0=OP.add)

            # ---------- cost ----------
            DTMP = sb.tile([TILE, T], F32, tag="DTMP")
            D2T = sb.tile([TILE, T], F32, tag="D2T")
            first = True
            for c in range(3):
                nc.vector.tensor_tensor(out=DTMP[:], in0=comp(TRR, c, T),
                                        in1=comp(PRJ, c, T), op=OP.subtract)
                nc.vector.tensor_tensor(out=DTMP[:], in0=DTMP[:], in1=DTMP[:], op=OP.mult)
                if first:
                    nc.vector.tensor_copy(out=D2T[:], in_=DTMP[:])
                    first = False
                else:
                    nc.vector.tensor_tensor(out=D2T[:], in0=D2T[:], in1=DTMP[:], op=OP.add)
            DIST = sb.tile([TILE, T], F32, tag="DIST")
            COST = sb.tile([TILE, 1], F32, tag="COST")
            nc.scalar.activation(out=DIST[:], in_=D2T[:], func=AF.Sqrt,
                                 accum_out=COST[:])

            # ---------- per-tile best-candidate selection ----------
            # costs to [4 samples, 32 cands] layout (q = d*16 + b)
            CBT = sb.tile([SPT, CPS], F32, tag="CBT")
            for d in range(2):
                nc.sync.dma_start(out=CBT[0:SPT, d * 16:(d + 1) * 16],
                                  in_=COST[d * 64:(d + 1) * 64, 0:1])
            MN4 = sb.tile([SPT, 1], F32, tag="MN4")
            nc.vector.tensor_reduce(out=MN4[:], in_=CBT[:], axis=mybir.AxisListType.X,
                                    op=OP.min)
            EQ4 = sb.tile([SPT, CPS], F32, tag="EQ4")
            nc.vector.tensor_scalar(out=EQ4[:], in0=CBT[:], scalar1=MN4[:],
                                    scalar2=None, op0=OP.is_equal)
            nc.vector.tensor_scalar(out=EQ4[:], in0=EQ4[:], scalar1=-BIG, scalar2=BIG,
                                    op0=OP.mult, op1=OP.add)
            nc.vector.tensor_tensor(out=EQ4[:], in0=EQ4[:], in1=io32_s[:], op=OP.add)
            IDXQ = sb.tile([SPT, 1], F32, tag="IDXQ")
            nc.vector.tensor_reduce(out=IDXQ[:], in_=EQ4[:], axis=mybir.AxisListType.X,
                                    op=OP.min)
            # replicate IDXQ to all 128 partitions: PE ones @ diag(IDXQ)
            DG = sb.tile([SPT, SPT], F32, tag="DG")
            nc.vector.tensor_scalar(out=DG[:], in0=i4_s[:], scalar1=IDXQ[:],
                                    scalar2=None, op0=OP.mult)
            IDXP = ps.tile([TILE, SPT], F32, tag="IDXP")
            nc.tensor.matmul(IDXP[:], lhsT=sel4_s[:], rhs=DG[:], start=True, stop=True)
            IDXR = sb.tile([TILE, SPT], F32, tag="IDXR")
            nc.vector.tensor_copy(out=IDXR[:], in_=IDXP[:])
            # one-hot [128, 4]: partition p selects sample s iff qp[p] == idx[s]
            OH = sb.tile([TILE, SPT], F32, tag="OH")
            nc.vector.tensor_scalar(out=OH[:], in0=IDXR[:], scalar1=qp_s[:],
                                    scalar2=None, op0=OP.is_equal)
            # restrict to the partition's own sample column
            nc.vector.tensor_tensor(out=OH[:], in0=OH[:], in1=selt_s[:], op=OP.mult)
            BPP = ps.tile([SPT, T * 3], F32, tag="BPP")
            nc.tensor.matmul(BPP[:], lhsT=OH[:], rhs=PRJ[:], start=True, stop=True)
            BPS = sb.tile([SPT, T * 3], F32, tag="BPS")
            nc.vector.tensor_copy(out=BPS[:], in_=BPP[:])
            nc.sync.dma_start(out=out[n0:n0 + SPT, :], in_=BPS[:])


_cached = {}


def _consts():
    p = np.arange(TILE)
    # partition p = d*64 + s*16 + b
    sel4 = ((p[None, :] % 64) // NB == np.arange(SPT)[:, None]).astype(np.float32)
    i4 = np.eye(SPT, dtype=np.float32)
    qp = ((p // 64) * NB + p % NB).astype(np.float32)[:, None]
    selt = sel4.T.copy()
    q = np.arange(CPS, dtype=np.float32)
    io32 = np.broadcast_to(q, (SPT, CPS)).copy()
    return dict(sel4=sel4, i4=i4, qp=qp, selt=selt, io32=io32)


def kernel(selected_traj, road_points, road_mask):
    selected_traj = np.asarray(selected_traj)
    road_points = np.asarray(road_points)
    road_mask = np.asarray(road_mask)

    if "nc" not in _cached:
        _cached["nc"] = build_program()
    nc = _cached["nc"]

    consts = _consts()
    in_maps = []
    for c in range(NCORES):
        sl = slice(c * NS, (c + 1) * NS)
        m = {
            "rp": np.ascontiguousarray(road_points[sl], dtype=np.float32),
            "msk": np.ascontiguousarray(road_mask[sl]).astype(np.uint8),
            "tr": np.ascontiguousarray(selected_traj[sl, :, 0:3], dtype=np.float32),
        }
        m.update(consts)
        in_maps.append(m)

    res = run_bass_kernel_spmd(nc, in_maps, list(range(NCORES)),
                               trace=bool(_cached.get("trace", False)))
    _cached["exec_time_ns"] = getattr(res, "exec_time_ns", None)
    outs = [np.asarray(res.results[c]["out"]).reshape(NS, T, 3) for c in range(NCORES)]
    out_pos = np.concatenate(outs, axis=0)

    if selected_traj.shape[-1] > 3:
        out_full = np.concatenate([out_pos, selected_traj[..., 3:]], axis=-1)
    else:
        out_full = out_pos
    return out_full.astype(selected_traj.dtype)

